# revision 1
# baseline (speedup 1.0000x reference)
"""DeepseekV3 decoder layer on 8 Trainium2 NeuronCores (Bass/Tile).

Sharding: tensor-parallel heads for MLA (2 heads/core), expert-parallel
routed experts (1/core), token shards (256 tok/core) for everything else.
Collectives: AllGather(latents) -> AllToAll(attn out) -> AllGather(h bf16)
+ AllGather(router weights) -> ReduceScatter(expert partials, bf16).

Activations kept feature-major [feat_p, tok_f]; weights pre-transposed on
host to [K, M] so each matmul is lhsT[K,M].T @ rhs[K,N], K = contraction.
Matmuls run float32r except the router chain (fp32) and experts (bf16).
"""
import sys

if "/opt/trn_rl_repo" not in sys.path:
    sys.path.insert(0, "/opt/trn_rl_repo")

import numpy as np
import ml_dtypes

import concourse.bass as bass
import concourse.bacc as bacc
import concourse.tile as tile
from concourse import mybir
from concourse import bass_utils

FP = mybir.dt.float32
BF = mybir.dt.bfloat16
FR = mybir.dt.float32r
AF = mybir.ActivationFunctionType
ALU = mybir.AluOpType

NCORE = 8
B, S, H = 2, 1024, 2048
T = B * S
NH, DN, DR, DV = 16, 128, 64, 128
DQK = DN + DR
KVL, QL = 512, 1536
NE, NG, INTER = 8, 4, 768
TSH = T // NCORE          # 256
HPC = NH // NCORE         # 2
SCALING = float(DQK) ** -0.5
RSF = 2.5
EPS = 1e-6

AG1_ROWS = QL + KVL + DR + 2   # 2114


def fr(ap):
    return ap.bitcast(FR)


def build_program():
    nc = bacc.Bacc("TRN2", target_bir_lowering=False, debug=False,
                   num_devices=NCORE)

    def din(name, shape, dtype=FP):
        return nc.dram_tensor(name, shape, dtype, kind="ExternalInput").ap()

    hidT = din("hidT", [H, TSH])
    qa_wT = din("qa_wT", [H, QL])
    kva_wT = din("kva_wT", [H, KVL + DR])
    qb_wT = din("qb_wT", [QL, HPC * DQK])      # cols: h0n h1n h0A h0B h1A h1B
    kvb_wT = din("kvb_wT", [KVL, HPC * 256])   # cols: k0 k1 v0 v1
    o_wT = din("o_wT", [NH * DV, H])
    r_wT = din("r_wT", [H, NE])
    r_bias = din("r_bias", [NE, 1])
    onehot = din("onehot", [NE, 1])
    g_wT = din("g_wT", [H, INTER], BF)
    u_wT = din("u_wT", [H, INTER], BF)
    d_wT = din("d_wT", [INTER, H], BF)
    sg_wT = din("sg_wT", [H, INTER])
    su_wT = din("su_wT", [H, INTER])
    sd_wT = din("sd_wT", [INTER, H])
    cc_q = din("cc_q", [128, T])
    ss_q = din("ss_q", [128, T])
    cc_k = din("cc_k", [DR, TSH])
    ss_k = din("ss_k", [DR, TSH])
    maskT_d = din("maskT", [512, 512])
    Gm_d = din("Gm", [NE, NG])
    Dg_d = din("Dg", [NG, NG * NG])
    Rg_d = din("Rg", [NG * NG, NG])
    Em_d = din("Em", [NG, NE])
    De_d = din("De", [NE, NE * NE])
    Re_d = din("Re", [NE * NE, NE])

    out = nc.dram_tensor("out", [H, TSH], FP, kind="ExternalOutput").ap()

    RG = [list(range(NCORE))]

    def dma(out_ap, in_ap):
        nc.sync.dma_start(out_ap, in_ap)

    def kp(ap, p=128):
        return ap.rearrange("(k p) t -> p k t", p=p)

    tcx = tile.TileContext(nc)
    tc = tcx.__enter__()
    dram_cm = tc.tile_pool(name="dram", bufs=1, space="DRAM")
    dram = dram_cm.__enter__()
    pp_cm = tc.tile_pool(name="persist", bufs=1)
    pp = pp_cm.__enter__()

    ag1_in = dram.tile([AG1_ROWS, TSH], FP)
    ag1_out = dram.tile([NCORE * AG1_ROWS, TSH], FP, addr_space="Shared")
    a2a_in = dram.tile([NCORE * 256, TSH], FP)
    a2a_out = dram.tile([NCORE * 256, TSH], FP)
    ag2_in = dram.tile([H, TSH], BF)
    ag2_out = dram.tile([NCORE * H, TSH], BF, addr_space="Shared")
    agw_in = dram.tile([NE, TSH], FP)
    agw_out = dram.tile([NCORE * NE, TSH], FP, addr_space="Shared")
    rs_in = dram.tile([NCORE * H, TSH], BF)
    rs_out = dram.tile([H, TSH], BF)

    ones = pp.tile([128, 1], FP)
    nc.vector.memset(ones[:], 1.0)
    epsb = pp.tile([128, 1], FP)
    nc.vector.memset(epsb[:], EPS)

    ag1v = ag1_out.rearrange("(j r) t -> j r t", r=AG1_ROWS)

    # ==================== phase A: local latents ====================
    with tc.tile_pool(name="pA", bufs=1) as pa, \
         tc.tile_pool(name="pAw", bufs=3) as paw, \
         tc.tile_pool(name="pAt", bufs=2) as pat, \
         tc.tile_pool(name="psA", bufs=2, space="PSUM") as psa:

        x0 = pa.tile([128, 16, TSH], FR)
        dma(x0[:], kp(hidT).bitcast(FR))

        ss_ps = psa.tile([1, TSH], FP, tag="st")
        for k in range(16):
            sq = pat.tile([128, TSH], FR, tag="sq")
            nc.scalar.square(sq[:], x0[:, k, :])
            nc.tensor.matmul(ss_ps[:], fr(ones[:]), fr(sq[:]),
                             start=(k == 0), stop=(k == 15))
        rstd = pa.tile([1, TSH], FP)
        nc.scalar.activation(rstd[:], ss_ps[:], AF.Sqrt,
                             bias=epsb[0:1, :], scale=1.0 / H)
        nc.vector.reciprocal(rstd[:], rstd[:])
        bloc = pa.tile([128, TSH], FP)
        nc.gpsimd.partition_broadcast(bloc[:], rstd[:1, :])

        qa_s = pa.tile([128, 12, TSH], FP)
        for m in range(12):
            wa = paw.tile([128, 16, 128], FR, tag="wa")
            dma(wa[:], kp(qa_wT[:, 128 * m:128 * (m + 1)]).bitcast(FR))
            ps = psa.tile([128, TSH], FP, tag="mm")
            for k in range(16):
                nc.tensor.matmul(ps[:], fr(wa[:, k, :]), fr(x0[:, k, :]),
                                 start=(k == 0), stop=(k == 15))
            nc.vector.tensor_mul(qa_s[:, m, :], ps[:], bloc[:])

        ss2 = psa.tile([1, TSH], FP, tag="st")
        for m in range(12):
            sq = pat.tile([128, TSH], FR, tag="sq")
            nc.scalar.square(sq[:], qa_s[:, m, :])
            nc.tensor.matmul(ss2[:], fr(ones[:]), fr(sq[:]),
                             start=(m == 0), stop=(m == 11))
        r2 = pa.tile([1, TSH], FP)
        nc.scalar.activation(r2[:], ss2[:], AF.Sqrt,
                             bias=epsb[0:1, :], scale=1.0 / QL)
        nc.vector.reciprocal(r2[:], r2[:])

        ckv_s = pa.tile([128, 4, TSH], FP)
        kr_raw = pa.tile([64, TSH], FP)
        for m in range(5):
            mc = 128 if m < 4 else 64
            wv = paw.tile([128, 16, 128], FR, tag="wa")
            dma(wv[:, :, :mc], kp(kva_wT[:, 128 * m:128 * m + mc]).bitcast(FR))
            ps = psa.tile([128, TSH], FP, tag="mm")
            for k in range(16):
                nc.tensor.matmul(ps[:mc, :], fr(wv[:, k, :mc]),
                                 fr(x0[:, k, :]),
                                 start=(k == 0), stop=(k == 15))
            if m < 4:
                nc.vector.tensor_mul(ckv_s[:, m, :], ps[:], bloc[:])
            else:
                nc.vector.tensor_mul(kr_raw[:], ps[:64, :], bloc[:64, :])

        ss3 = psa.tile([1, TSH], FP, tag="st")
        for m in range(4):
            sq = pat.tile([128, TSH], FR, tag="sq")
            nc.scalar.square(sq[:], ckv_s[:, m, :])
            nc.tensor.matmul(ss3[:], fr(ones[:]), fr(sq[:]),
                             start=(m == 0), stop=(m == 3))
        r3 = pa.tile([1, TSH], FP)
        nc.scalar.activation(r3[:], ss3[:], AF.Sqrt,
                             bias=epsb[0:1, :], scale=1.0 / KVL)
        nc.vector.reciprocal(r3[:], r3[:])

        # local k rope (rot rows pre-permuted to [A(32) B(32)] on host)
        cck = pa.tile([64, TSH], FP)
        ssk = pa.tile([64, TSH], FP)
        dma(cck[:], cc_k[:])
        dma(ssk[:], ss_k[:])
        kr_sh = pa.tile([64, TSH], FP)
        dma(kr_sh[0:32, :], kr_raw[32:64, :])
        dma(kr_sh[32:64, :], kr_raw[0:32, :])
        nc.vector.tensor_mul(kr_sh[:], kr_sh[:], ssk[:])
        kr = pa.tile([64, TSH], FP)
        nc.vector.tensor_mul(kr[:], kr_raw[:], cck[:])
        nc.vector.tensor_add(kr[:], kr[:], kr_sh[:])

        dma(ag1_in[0:QL, :].rearrange("(m p) t -> p m t", p=128), qa_s[:])
        dma(ag1_in[QL:QL + KVL, :].rearrange("(m p) t -> p m t", p=128),
            ckv_s[:])
        dma(ag1_in[QL + KVL:QL + KVL + DR, :], kr[:])
        dma(ag1_in[2112:2113, :], r2[:])
        dma(ag1_in[2113:2114, :], r3[:])

    nc.gpsimd.collective_compute(
        "AllGather", ALU.bypass, replica_groups=RG,
        ins=[ag1_in.opt()], outs=[ag1_out.opt()])

    # ==================== attention ====================
    with tc.tile_pool(name="att", bufs=1) as at, \
         tc.tile_pool(name="atp", bufs=2) as atp, \
         tc.tile_pool(name="psT", bufs=2, space="PSUM") as pst:

        qn = at.tile([128, 2, T], FR)
        qr = at.tile([128, T], FR)
        qr1 = at.tile([64, T], FR)
        kn = at.tile([128, 2, T], FR)
        krotg = at.tile([64, T], FR)
        vt = at.tile([128, 16, TSH], FR)
        attn = at.tile([128, 2, T], FP)
        maskT = at.tile([128, 4, 512], FP)
        dma(maskT[:], kp(maskT_d))

        with tc.tile_pool(name="proj", bufs=1) as pj, \
             tc.tile_pool(name="projs", bufs=2) as pjs:

            qb_sb = pj.tile([128, 12, HPC * DQK], FR)
            dma(qb_sb[:], kp(qb_wT).bitcast(FR))
            kvb_sb = pj.tile([128, 4, HPC * 256], FR)
            dma(kvb_sb[:], kp(kvb_wT).bitcast(FR))

            b2 = pj.tile([128, T], FP)
            dma(b2[0:1, :], ag1v[:, 2112:2113, :].rearrange("j a t -> a j t"))
            nc.gpsimd.partition_broadcast(b2[:], b2[0:1, :])
            b3 = pj.tile([128, T], FP)
            dma(b3[0:1, :], ag1v[:, 2113:2114, :].rearrange("j a t -> a j t"))
            nc.gpsimd.partition_broadcast(b3[:], b3[0:1, :])
            r3T = pj.tile([128, 16], FP)
            for _n in range(NCORE):
                for _s in range(2):
                    dma(r3T[:, 2 * _n + _s:2 * _n + _s + 1],
                        ag1v[_n, 2113:2114,
                             128 * _s:128 * (_s + 1)].rearrange(
                                 "a t -> t a"))

            dma(krotg[:].rearrange("p (j t) -> p j t", t=TSH),
                ag1v[:, QL + KVL:QL + KVL + DR, :].rearrange(
                    "j p t -> p j t").bitcast(FR))

            for n in range(NCORE):          # 256-token tiles
                nsl = slice(TSH * n, TSH * (n + 1))
                qrhs = pjs.tile([128, 12, TSH], FR, tag="qrhs")
                dma(qrhs[:], ag1v[n, 0:QL, :].rearrange(
                    "(k p) t -> p k t", p=128).bitcast(FR))
                qro = pjs.tile([128, TSH], FP, tag="qro")
                for m in range(3):
                    ps = pst.tile([128, TSH], FP, tag="mm")
                    for k in range(12):
                        nc.tensor.matmul(
                            ps[:], fr(qb_sb[:, k, 128 * m:128 * (m + 1)]),
                            fr(qrhs[:, k, :]),
                            start=(k == 0), stop=(k == 11))
                    dst = qn[:, m, nsl] if m < 2 else qro[:]
                    nc.vector.tensor_mul(dst, ps[:], b2[:, nsl])
                # rope this token tile
                qsh = pjs.tile([128, TSH], FP, tag="qsh")
                dma(qsh[0:32, :], qro[32:64, :])
                dma(qsh[32:64, :], qro[0:32, :])
                dma(qsh[64:96, :], qro[96:128, :])
                dma(qsh[96:128, :], qro[64:96, :])
                ccn = pjs.tile([128, TSH], FP, tag="ccn")
                dma(ccn[:], cc_q[:, nsl])
                ssn = pjs.tile([128, TSH], FP, tag="ssn")
                dma(ssn[:], ss_q[:, nsl])
                nc.vector.tensor_mul(qsh[:], qsh[:], ssn[:])
                nc.vector.tensor_mul(qr[:, nsl], qro[:], ccn[:])
                nc.vector.tensor_add(qr[:, nsl], qr[:, nsl], qsh[:])

                # kv_b for this token tile
                lat_n = pjs.tile([128, 4, TSH], FR, tag="latn")
                dma(lat_n[:], ag1v[n, QL:QL + KVL, :].rearrange(
                    "(k p) t -> p k t", p=128).bitcast(FR))
                for h in range(2):
                    ps = pst.tile([128, TSH], FP, tag="mm")
                    for k in range(4):
                        nc.tensor.matmul(
                            ps[:], fr(kvb_sb[:, k, 128 * h:128 * (h + 1)]),
                            fr(lat_n[:, k, :]),
                            start=(k == 0), stop=(k == 3))
                    nc.vector.tensor_mul(kn[:, h, nsl], ps[:], b3[:, nsl])
                for s2 in range(2):
                    ps = pst.tile([128, TSH], FP, tag="mm")
                    for k in range(4):
                        nc.tensor.matmul(
                            ps[:], fr(lat_n[:, k, 128 * s2:128 * (s2 + 1)]),
                            fr(kvb_sb[:, k, 256:512]),
                            start=(k == 0), stop=(k == 3))
                    sp = 2 * n + s2
                    nc.vector.tensor_scalar(vt[:, sp, :], ps[:],
                                            r3T[:, sp:sp + 1], None,
                                            ALU.mult)

        dma(qr1[:], qr[64:128, :])

        # flash attention, scores transposed [s'_p, s_f]
        for b_ in range(2):
            for h in range(2):
                for sqi in range(2):
                    q0 = 1024 * b_ + 512 * sqi
                    qsl = slice(q0, q0 + 512)
                    nk = 4 * (sqi + 1)
                    aps = pst.tile([128, 512], FP, tag="av")
                    dps = pst.tile([1, 512], FP, tag="dn")
                    for sk in range(nk):
                        k0 = 1024 * b_ + 128 * sk
                        ksl = slice(k0, k0 + 128)
                        sps = pst.tile([128, 512], FP, tag="sc")
                        nc.tensor.matmul(sps[:], fr(kn[:, h, ksl]),
                                         fr(qn[:, h, qsl]),
                                         start=True, stop=False)
                        qrh = qr[0:64, qsl] if h == 0 else qr1[:, qsl]
                        nc.tensor.matmul(
                            sps[:], fr(krotg[:, ksl]), fr(qrh),
                            start=False, stop=True)
                        pr = atp.tile([128, 512], FR, tag="pr", bufs=2)
                        nc.scalar.activation(pr[:], sps[:], AF.Exp,
                                             scale=SCALING)
                        if sk >= 4 * sqi:
                            nc.vector.tensor_mul(
                                pr[:], pr[:], maskT[:, sk - 4 * sqi, :])
                        nc.tensor.matmul(
                            aps[:], fr(vt[:, 8 * b_ + sk,
                                          128 * h:128 * (h + 1)]),
                            fr(pr[:]), start=(sk == 0),
                            stop=(sk == nk - 1), skip_group_check=True)
                        nc.tensor.matmul(
                            dps[:], fr(ones[:]), fr(pr[:]),
                            start=(sk == 0), stop=(sk == nk - 1),
                            skip_group_check=True)
                    rd = atp.tile([1, 512], FP, tag="rd", bufs=1)
                    nc.vector.reciprocal(rd[:], dps[:])
                    rdb = atp.tile([128, 512], FP, tag="rdb", bufs=1)
                    nc.gpsimd.partition_broadcast(rdb[:], rd[:1, :])
                    nc.vector.tensor_mul(attn[:, h, qsl], aps[:], rdb[:])

        a2av = a2a_in.rearrange("(j h p) t -> j p h t", h=2, p=128)
        for j in range(NCORE):
            dma(a2av[j], attn[:, :, TSH * j:TSH * (j + 1)])

    nc.gpsimd.collective_compute(
        "AllToAll", ALU.bypass, replica_groups=RG,
        ins=[a2a_in.opt()], outs=[a2a_out.opt()])

    # ==================== o_proj + ln2 + router ====================
    late_cm = tc.tile_pool(name="late", bufs=1)
    late = late_cm.__enter__()
    x2s = late.tile([128, 16, TSH], FP)
    hs = late.tile([128, 16, TSH], FR)
    bce = late.tile([128, T], FP)

    with tc.tile_pool(name="op", bufs=1) as po, \
         tc.tile_pool(name="opw", bufs=3) as pow_, \
         tc.tile_pool(name="opt", bufs=2) as pot, \
         tc.tile_pool(name="psO", bufs=2, space="PSUM") as pso:

        x0r = po.tile([128, 16, TSH], FP)
        dma(x0r[:], kp(hidT))
        attn_sb = po.tile([128, 16, TSH], FR)
        dma(attn_sb[:], kp(a2a_out[:, :]).bitcast(FR))

        for m in range(16):
            ow = pow_.tile([128, 16, 128], FR, tag="ow")
            dma(ow[:], kp(o_wT[:, 128 * m:128 * (m + 1)]).bitcast(FR))
            ps = pso.tile([128, TSH], FP, tag="mm")
            for k in range(16):
                nc.tensor.matmul(ps[:], fr(ow[:, k, :]),
                                 fr(attn_sb[:, k, :]),
                                 start=(k == 0), stop=(k == 15))
            nc.vector.tensor_add(x2s[:, m, :], ps[:], x0r[:, m, :])

        ss4 = pso.tile([1, TSH], FP, tag="st")
        for k in range(16):
            sq = pot.tile([128, TSH], FR, tag="sq")
            nc.scalar.square(sq[:], x2s[:, k, :])
            nc.tensor.matmul(ss4[:], fr(ones[:]), fr(sq[:]),
                             start=(k == 0), stop=(k == 15))
        r4 = po.tile([1, TSH], FP)
        nc.scalar.activation(r4[:], ss4[:], AF.Sqrt,
                             bias=epsb[0:1, :], scale=1.0 / H)
        nc.vector.reciprocal(r4[:], r4[:])
        b4 = po.tile([128, TSH], FP)
        nc.gpsimd.partition_broadcast(b4[:], r4[:1, :])
        hb = po.tile([128, 16, TSH], BF)
        for m in range(16):
            nc.vector.tensor_mul(hs[:, m, :], x2s[:, m, :], b4[:])
            nc.scalar.copy(hb[:, m, :], hs[:, m, :])
        dma(ag2_in[:, :].rearrange("(m p) t -> p m t", p=128), hb[:])

        # router (fp32 matmuls)
        rw_sb = po.tile([128, 16, NE], FP)
        dma(rw_sb[:], kp(r_wT))
        rb_sb = po.tile([NE, 1], FP)
        dma(rb_sb[:], r_bias[:])
        Gm_s = po.tile([NE, NG], FP)
        dma(Gm_s[:], Gm_d[:])
        Dg_s = po.tile([NG, 16], FP)
        dma(Dg_s[:], Dg_d[:])
        Rg_s = po.tile([16, NG], FP)
        dma(Rg_s[:], Rg_d[:])
        Em_s = po.tile([NG, NE], FP)
        dma(Em_s[:], Em_d[:])
        De_s = po.tile([NE, 64], FP)
        dma(De_s[:], De_d[:])
        Re_s = po.tile([64, NE], FP)
        dma(Re_s[:], Re_d[:])

        lg = pso.tile([NE, TSH], FP, tag="rt")
        for k in range(16):
            nc.tensor.matmul(lg[:], rw_sb[:, k, :], hs[:, k, :].bitcast(FP),
                             start=(k == 0), stop=(k == 15))
        sr = po.tile([NE, TSH], FP)
        nc.scalar.activation(sr[:], lg[:], AF.Sigmoid)
        sc_t = po.tile([NE, TSH], FP)
        nc.vector.tensor_scalar(sc_t[:], sr[:], rb_sb[:, 0:1], None, ALU.add)
        gs_ps = pso.tile([NG, TSH], FP, tag="rt")
        nc.tensor.matmul(gs_ps[:], Gm_s[:], sc_t[:])
        gs_sb = po.tile([NG, TSH], FP)
        nc.scalar.copy(gs_sb[:], gs_ps[:])
        gd_ps = pso.tile([16, TSH], FP, tag="rt")
        nc.tensor.matmul(gd_ps[:], Dg_s[:], gs_sb[:])
        gp = po.tile([16, TSH], FP)
        nc.vector.tensor_scalar(gp[:], gd_ps[:], 0.0, None, ALU.is_gt)
        gc_ps = pso.tile([NG, TSH], FP, tag="rt")
        nc.tensor.matmul(gc_ps[:], Rg_s[:], gp[:])
        gm = po.tile([NG, TSH], FP)
        nc.vector.tensor_scalar(gm[:], gc_ps[:], 2.0, None, ALU.is_lt)
        em_ps = pso.tile([NE, TSH], FP, tag="rt")
        nc.tensor.matmul(em_ps[:], Em_s[:], gm[:])
        msk = po.tile([NE, TSH], FP)
        nc.vector.tensor_mul(msk[:], em_ps[:], sc_t[:])
        ed_ps = pso.tile([64, TSH], FP, tag="rt")
        nc.tensor.matmul(ed_ps[:], De_s[:], msk[:])
        ep = po.tile([64, TSH], FP)
        nc.vector.tensor_scalar(ep[:], ed_ps[:], 0.0, None, ALU.is_gt)
        ec_ps = pso.tile([NE, TSH], FP, tag="rt")
        nc.tensor.matmul(ec_ps[:], Re_s[:], ep[:])
        es = po.tile([NE, TSH], FP)
        nc.vector.tensor_scalar(es[:], ec_ps[:], 2.0, None, ALU.is_lt)
        w_sb = po.tile([NE, TSH], FP)
        nc.vector.tensor_mul(w_sb[:], es[:], sr[:])
        ws_ps = pso.tile([1, TSH], FP, tag="rt")
        nc.tensor.matmul(ws_ps[:], ones[0:NE, :], w_sb[:])
        wse = po.tile([1, TSH], FP)
        nc.vector.tensor_scalar(wse[:], ws_ps[:], 1e-20, None, ALU.add)
        nc.vector.reciprocal(wse[:], wse[:])
        wb = po.tile([NE, TSH], FP)
        nc.gpsimd.partition_broadcast(wb[:], wse[:1, :])
        dw_sb = po.tile([NE, TSH], FP)
        nc.vector.scalar_tensor_tensor(dw_sb[:], w_sb[:], RSF, wb[:],
                                       ALU.mult, ALU.mult)
        dma(agw_in[:, :], dw_sb[:])

        nc.gpsimd.collective_compute(
            "AllGather", ALU.bypass, replica_groups=RG,
            ins=[ag2_in.opt()], outs=[ag2_out.opt()])
        nc.gpsimd.collective_compute(
            "AllGather", ALU.bypass, replica_groups=RG,
            ins=[agw_in.opt()], outs=[agw_out.opt()])

        oh_sb = po.tile([NE, 1], FP)
        dma(oh_sb[:], onehot[:])
        dwg = po.tile([NE, NCORE, TSH], FP)
        dma(dwg[:], agw_out[:, :].rearrange("(j p) t -> p j t", p=NE))
        for jj in range(4):
            ewp = pso.tile([1, 512], FP, tag="rt")
            for q in range(2):
                nc.tensor.matmul(ewp[:, TSH * q:TSH * (q + 1)],
                                 oh_sb[:], dwg[:, 2 * jj + q, :])
            nc.scalar.copy(bce[0:1, 512 * jj:512 * (jj + 1)], ewp[:])
        nc.gpsimd.partition_broadcast(bce[:], bce[0:1, :])

    # ==================== MoE (bf16) + shared expert ====================
    ag2v = ag2_out.rearrange("(j r) t -> j r t", r=H)
    with tc.tile_pool(name="moe", bufs=1) as pm, \
         tc.tile_pool(name="moet", bufs=2) as pmt, \
         tc.tile_pool(name="moew", bufs=2) as pmw, \
         tc.tile_pool(name="psM", bufs=2, space="PSUM") as psm:

        gw_sb = pm.tile([128, 16, INTER], BF)
        dma(gw_sb[:], kp(g_wT))
        uw_sb = pm.tile([128, 16, INTER], BF)
        dma(uw_sb[:], kp(u_wT))
        dwn_sb = pm.tile([128, 6, H], BF)
        dma(dwn_sb[:], kp(d_wT))

        rsv = rs_in.rearrange("(j m p) t -> j m p t", m=16, p=128)
        for n in range(4):
            nsl = slice(512 * n, 512 * (n + 1))
            hb_n = pmt.tile([128, 16, 2, TSH], BF, tag="hb", bufs=1)
            for jj in range(2):
                dma(hb_n[:, :, jj, :],
                    ag2v[2 * n + jj].rearrange("(k p) t -> p k t", p=128))
            act_n = pmt.tile([128, 6, 512], BF, tag="act")
            for m in range(6):
                gp_ = psm.tile([128, 512], FP, tag="mg")
                for k in range(16):
                    nc.tensor.matmul(gp_[:],
                                     gw_sb[:, k, 128 * m:128 * (m + 1)],
                                     hb_n[:, k, :, :],
                                     start=(k == 0), stop=(k == 15))
                gsi = pmt.tile([128, 512], FP, tag="gsi")
                nc.scalar.activation(gsi[:], gp_[:], AF.Sigmoid)
                nc.vector.tensor_mul(gsi[:], gp_[:], gsi[:])
                up_ = psm.tile([128, 512], FP, tag="mg")
                for k in range(16):
                    nc.tensor.matmul(up_[:],
                                     uw_sb[:, k, 128 * m:128 * (m + 1)],
                                     hb_n[:, k, :, :],
                                     start=(k == 0), stop=(k == 15))
                nc.vector.tensor_mul(act_n[:, m, :], up_[:], gsi[:])
            for m in range(16):
                dp = psm.tile([128, 512], FP, tag="md")
                for k in range(6):
                    nc.tensor.matmul(dp[:],
                                     dwn_sb[:, k, 128 * m:128 * (m + 1)],
                                     act_n[:, k, :],
                                     start=(k == 0), stop=(k == 5))
                eo = pmw.tile([128, 512], BF, tag="eo", bufs=3)
                nc.vector.tensor_mul(eo[:], dp[:], bce[:, nsl])
                dma(rsv[2 * n, m], eo[:, 0:TSH])
                dma(rsv[2 * n + 1, m], eo[:, TSH:512])

        # shared expert (f32r, token shard)
        act2 = pm.tile([128, 6, TSH], FR)
        for m in range(6):
            sgw = pmw.tile([128, 16, 128], FR, tag="sgw")
            dma(sgw[:], kp(sg_wT[:, 128 * m:128 * (m + 1)]).bitcast(FR))
            g2 = psm.tile([128, 512], FP, tag="mg")
            for k in range(16):
                nc.tensor.matmul(g2[:, 0:TSH], fr(sgw[:, k, :]),
                                 fr(hs[:, k, :]),
                                 start=(k == 0), stop=(k == 15))
            g2s = pmt.tile([128, TSH], FP, tag="g2s")
            nc.scalar.activation(g2s[:], g2[:, 0:TSH], AF.Sigmoid)
            nc.vector.tensor_mul(g2s[:], g2[:, 0:TSH], g2s[:])
            suw = pmw.tile([128, 16, 128], FR, tag="sgw")
            dma(suw[:], kp(su_wT[:, 128 * m:128 * (m + 1)]).bitcast(FR))
            u2 = psm.tile([128, 512], FP, tag="mg")
            for k in range(16):
                nc.tensor.matmul(u2[:, 0:TSH], fr(suw[:, k, :]),
                                 fr(hs[:, k, :]),
                                 start=(k == 0), stop=(k == 15))
            nc.vector.tensor_mul(act2[:, m, :], u2[:, 0:TSH], g2s[:])
        for m in range(16):
            sdw = pmw.tile([128, 6, 128], FR, tag="sdw")
            dma(sdw[:], kp(sd_wT[:, 128 * m:128 * (m + 1)]).bitcast(FR))
            d2 = psm.tile([128, 512], FP, tag="md")
            for k in range(6):
                nc.tensor.matmul(d2[:, 0:TSH], fr(sdw[:, k, :]),
                                 fr(act2[:, k, :]),
                                 start=(k == 0), stop=(k == 5))
            nc.vector.tensor_add(x2s[:, m, :], d2[:, 0:TSH], x2s[:, m, :])

        nc.gpsimd.collective_compute(
            "ReduceScatter", ALU.add, replica_groups=RG,
            ins=[rs_in.opt()], outs=[rs_out.opt()])

        for m in range(16):
            rsb = pmt.tile([128, TSH], BF, tag="rsb")
            dma(rsb[:], kp(rs_out[:, :])[:, m, :])
            fin = pmt.tile([128, TSH], FP, tag="fin")
            nc.vector.tensor_add(fin[:], rsb[:], x2s[:, m, :])
            dma(out[128 * m:128 * (m + 1), :], fin[:])

    late_cm.__exit__(None, None, None)
    pp_cm.__exit__(None, None, None)
    dram_cm.__exit__(None, None, None)
    tcx.__exit__(None, None, None)

    nc.compile()
    return nc


# --------------------------------------------------------------------------
# host side
# --------------------------------------------------------------------------

_PERM64 = np.concatenate([np.arange(0, 64, 2), np.arange(1, 64, 2)])


def _routing_mats():
    Gm = np.zeros((NE, NG), np.float32)
    for g in range(NG):
        Gm[2 * g, g] = 1.0
        Gm[2 * g + 1, g] = 1.0
    Dg = np.zeros((NG, NG * NG), np.float32)
    Rg = np.zeros((NG * NG, NG), np.float32)
    for i in range(NG):
        for j in range(NG):
            p = i * NG + j
            Dg[i, p] += 1.0
            Dg[j, p] -= 1.0
            Rg[p, j] = 1.0
    Em = np.zeros((NG, NE), np.float32)
    for g in range(NG):
        Em[g, 2 * g] = 1.0
        Em[g, 2 * g + 1] = 1.0
    De = np.zeros((NE, NE * NE), np.float32)
    Re = np.zeros((NE * NE, NE), np.float32)
    for i in range(NE):
        for j in range(NE):
            p = i * NE + j
            De[i, p] += 1.0
            De[j, p] -= 1.0
            Re[p, j] = 1.0
    return Gm, Dg, Rg, Em, De, Re


def _c(a):
    return np.ascontiguousarray(a, dtype=np.float32)


def _bfc(a):
    return np.ascontiguousarray(np.asarray(a, np.float32).astype(
        ml_dtypes.bfloat16))


def make_in_maps(inputs):
    f32 = np.float32
    hs_ = np.asarray(inputs["hidden_states"], f32).reshape(T, H)
    cos = np.asarray(inputs["cos"], f32).reshape(T, DR)
    sin = np.asarray(inputs["sin"], f32).reshape(T, DR)
    ln1 = np.asarray(inputs["ln1_w"], f32)
    ln2 = np.asarray(inputs["ln2_w"], f32)
    qaln = np.asarray(inputs["q_a_ln_w"], f32)
    kvln = np.asarray(inputs["kv_a_ln_w"], f32)

    qa_w = np.asarray(inputs["q_a_w"], f32) * ln1[None, :]
    kva_w = np.asarray(inputs["kv_a_w"], f32) * ln1[None, :]
    kva_w = np.concatenate([kva_w[:KVL], kva_w[KVL:][_PERM64]], 0)
    qb_w = np.asarray(inputs["q_b_w"], f32) * qaln[None, :]
    kvb_w = np.asarray(inputs["kv_b_w"], f32) * kvln[None, :]
    o_w = np.asarray(inputs["o_w"], f32)
    r_w = np.asarray(inputs["router_w"], f32) * ln2[None, :]
    r_b = np.asarray(inputs["router_bias"], f32)
    g_w = np.asarray(inputs["gate_w"], f32) * ln2[None, None, :]
    u_w = np.asarray(inputs["up_w"], f32) * ln2[None, None, :]
    d_w = np.asarray(inputs["down_w"], f32)
    sg_w = np.asarray(inputs["sh_gate_w"], f32) * ln2[None, :]
    su_w = np.asarray(inputs["sh_up_w"], f32) * ln2[None, :]
    sd_w = np.asarray(inputs["sh_down_w"], f32)

    cosT = cos.T
    sinT = sin.T
    cc_q = np.concatenate([cosT[0:32], cosT[32:64]] * 2, 0)
    ss_q = np.concatenate([-sinT[0:32], sinT[32:64]] * 2, 0)
    maskT = np.triu(np.ones((512, 512), np.float32))
    Gm, Dg, Rg, Em, De, Re = _routing_mats()

    shared = dict(
        qa_wT=_c(qa_w.T), kva_wT=_c(kva_w.T), o_wT=_c(o_w.T),
        r_wT=_c(r_w.T), r_bias=_c(r_b.reshape(NE, 1)),
        sg_wT=_c(sg_w.T), su_wT=_c(su_w.T), sd_wT=_c(sd_w.T),
        cc_q=_c(cc_q), ss_q=_c(ss_q), maskT=_c(maskT),
        Gm=_c(Gm), Dg=_c(Dg), Rg=_c(Rg), Em=_c(Em), De=_c(De), Re=_c(Re),
    )

    in_maps = []
    for c in range(NCORE):
        tsl = slice(TSH * c, TSH * (c + 1))
        h0, h1 = 2 * c, 2 * c + 1
        qb_cols = [qb_w[DQK * h0:DQK * h0 + DN],
                   qb_w[DQK * h1:DQK * h1 + DN]]
        for h in (h0, h1):
            rot = qb_w[DQK * h + DN:DQK * (h + 1)]
            qb_cols.append(rot[0::2])
            qb_cols.append(rot[1::2])
        qb_c = np.concatenate(qb_cols, 0)              # [384, QL]
        kvb_c = np.concatenate(
            [kvb_w[256 * h0:256 * h0 + 128],
             kvb_w[256 * h1:256 * h1 + 128],
             kvb_w[256 * h0 + 128:256 * h0 + 256],
             kvb_w[256 * h1 + 128:256 * h1 + 256]], 0)  # [512, KVL]
        oh = np.zeros((NE, 1), np.float32)
        oh[c, 0] = 1.0
        m = dict(shared)
        m.update(
            hidT=_c(hs_[tsl].T),
            qb_wT=_c(qb_c.T), kvb_wT=_c(kvb_c.T),
            cc_k=_c(cosT[:, tsl]),
            ss_k=_c(np.concatenate([-sinT[0:32, tsl],
                                    sinT[32:64, tsl]], 0)),
            onehot=_c(oh),
            g_wT=_bfc(g_w[c].T), u_wT=_bfc(u_w[c].T), d_wT=_bfc(d_w[c].T),
        )
        in_maps.append(m)
    return in_maps


_NC_CACHE = None


def _get_nc():
    global _NC_CACHE
    if _NC_CACHE is None:
        _NC_CACHE = build_program()
    return _NC_CACHE


def kernel(**inputs) -> np.ndarray:
    nc = _get_nc()
    in_maps = make_in_maps(inputs)
    res = bass_utils.run_bass_kernel_spmd(nc, in_maps,
                                          core_ids=list(range(NCORE)))
    full = np.empty((H, T), np.float32)
    for c in range(NCORE):
        full[:, TSH * c:TSH * (c + 1)] = res.results[c]["out"]
    return np.ascontiguousarray(full.T).reshape(B, S, H)

